# revision 37
# baseline (speedup 1.0000x reference)
"""Trainium2 Bass kernel for nn_GameTensor_27195732918735.

Computes out[i,j,b] = Hessian_z V_i(z_all[j,b]) for i != j, zeros on the
diagonal, where V_i(z) = W2[i] @ tanh(W1[i] @ z + b1[i]) + b2[i].

Analytic form used on-device:
    u = W1 z + b1;  th = tanh(u);  s_k = th_k (1 - th_k^2)
    H = sum_k s_k (-2 W2_k w1_k) w1_k^T

H is symmetric, so the device only computes the packed half: columns
(a, c=(a+t) mod 128) for t = 0..64, i.e. P = 65*128 = 8320 of the 16384
(a,c) cells. The host mirrors the packed half into the full symmetric
matrix during unshard (pure gather, no arithmetic).

Per-core plan (8 cores, SPMD): core c owns agent i = c//2 and three
(j, batch-half) tasks. On-chip, packed T[k, t, a] = (-2 W2 W1)[k,a] *
W1[k,(a+t)%128] is built with 9 wide fp16 DVE ops (2x mode) using a
sliding-window access pattern over a duplicated-W1 tile, then each task is
a [k=256] x [b=128] x [P] fp16 matmul. PSUM->SBUF fp16 drain is split
Scalar/Vector; two groups per task plus the tail bypass the drain and DMA
fp32 straight from PSUM. Dummy matmuls at kernel start keep the PE p-state
ramped. Output DMAs go per-group on the sync/gpsimd DGE queues.
"""

import numpy as np

import concourse.bass as bass
import concourse.mybir as mybir
import concourse.tile as tile
from concourse import bacc
from concourse.bass_utils import run_bass_kernel_spmd

N, B, D = 4, 256, 128
H2 = 2 * D  # 256 hidden
NCORES = 8
NTASK = 3  # (j, half) tasks per core
HALF = B // 2  # 128 batches per task
NT = 65  # packed diagonals t = 0..64
P = NT * D  # 8320 packed (t,a) columns
GROUP = 1024  # psum drain granularity (2 banks)
NGRP = P // GROUP  # 8 full groups per task
TAIL = P - NGRP * GROUP  # 128
VE_N = ({3, 5, 7}, {3, 5, 7}, {1, 4})  # per-task groups drained by Vector
NDUM = 4  # PE warm-up matmuls at kernel start (more emitted mid-stream)
# merged fp16 input block layout (columns); [w1t|zt] loads first (gates S)
O_W1T, O_ZT, O_W1D, O_W1S, NCOLS = 0, 256, 640, 1028, 1284

# matmul operand dtype: "fp16" (default) or "bf16". Other values fall back
# to fp16. Exists for test-harness A/B runs.
MM_MODE = "fp16"

_F32 = mybir.dt.float32


def _mmdt():
    return mybir.dt.bfloat16 if MM_MODE == "bf16" else mybir.dt.float16


def _window(ap_src, t0, tn, kdim, adim):
    """Sliding-window AP: w1d[:, :, None, t0:t0+adim] broadcast to
    [128, kdim, tn, adim], then the broadcast (stride-0) window dim is
    re-strided to 1 so element (kc, i, a) reads w1d[:, kc, t0+i+a]."""
    v = ap_src[:, :, None, t0 : t0 + adim].to_broadcast((128, kdim, tn, adim))
    v.ap[2] = (1, tn)
    return v


def _emit(tc, nc, inp, b1c, out16):
    mmdt = _mmdt()
    Tanh = mybir.ActivationFunctionType.Tanh
    Square = mybir.ActivationFunctionType.Square
    mult = mybir.AluOpType.mult
    add = mybir.AluOpType.add

    with (
        tc.tile_pool(name="consts", bufs=1) as consts,
        tc.tile_pool(name="tpool", bufs=1) as tpool,
        tc.tile_pool(name="small", bufs=4) as small,
        tc.tile_pool(name="stage", bufs=3) as stage_pool,
        tc.tile_pool(name="psum", bufs=4, space="PSUM") as psum,
    ):
        # ---- PE warm-up: ramp the p-state before real work is ready ---------
        dumw = consts.tile([128, 512], mmdt)
        nc.gpsimd.memset(dumw, 0.0)

        def emit_dummies(k):
            for _ in range(k):
                dps = psum.tile([128, GROUP], _F32, tag="ps")
                nc.tensor.matmul(
                    dps[:, :512], lhsT=dumw[:, :128], rhs=dumw, start=True, stop=True
                )

        emit_dummies(NDUM)

        # ---- load constants (S-path half first, T-path half second) ---------
        inp_sb = consts.tile([128, NCOLS], mmdt)
        nc.sync.dma_start(inp_sb[:, :O_W1D], inp[:, :O_W1D])
        nc.sync.dma_start(inp_sb[:, O_W1D:], inp[:, O_W1D:])
        b1_sb = consts.tile([128, 2], _F32)
        nc.scalar.dma_start(b1_sb, b1c)

        w1t_sb = inp_sb[:, O_W1T:O_ZT]
        zt_sb = inp_sb[:, O_ZT:O_W1D].rearrange("p (t b) -> p t b", t=NTASK)
        w1d_sb = inp_sb[:, O_W1D:O_W1S].rearrange("p (k a) -> p k a", k=2)
        w1s_sb = inp_sb[:, O_W1S:NCOLS].rearrange("p (k a) -> p k a", k=2)

        # ---- S[k, b] per task: s = th * (1 - th^2) --------------------------
        # (-2 W2 is folded into the T operand host-side.)
        s_sb = consts.tile([128, NTASK, 2, 128], mmdt)  # [k%128, task, kc, b]

        def emit_s(t):
            # task 0 gates the first main matmul; its elementwise tail runs on
            # the otherwise-idle GpSimd so it never queues behind Vector's T
            # chunks (both engine queues are in-order).
            eng = nc.gpsimd if t == 0 else nc.vector
            for kc in range(2):
                ups = psum.tile([128, GROUP], _F32, tag="ps")
                nc.tensor.matmul(
                    ups[:, :128],
                    lhsT=w1t_sb[:, kc * 128 : (kc + 1) * 128],
                    rhs=zt_sb[:, t, :],
                    start=True,
                    stop=True,
                )
                th = small.tile([128, 128], _F32, tag="th")
                nc.scalar.activation(th, ups[:, :128], Tanh, bias=b1_sb[:, kc : kc + 1])
                th2 = small.tile([128, 128], _F32, tag="th2")
                nc.scalar.activation(th2, th, Square)
                sm = small.tile([128, 128], _F32, tag="sm")
                eng.tensor_scalar(sm, th2, -1.0, 1.0, mult, add)
                eng.tensor_tensor(s_sb[:, t, kc, :], th, sm, mult)

        # ---- packed T[k, kc, t, a] = w1s[k,a] * W1[k,(a+t)%128] -------------
        TG = 8  # t-values per DVE op
        NCHUNK = (NT + TG - 1) // TG
        TT = tpool.tile([128, 2, NT, 128], mmdt)

        def emit_t_chunk(g):
            t0 = g * TG
            tn = min(TG, NT - t0)
            dst = TT[:, :, t0 : t0 + tn, :]
            in0 = w1s_sb[:, :, None, :].to_broadcast((128, 2, tn, 128))
            in1 = _window(w1d_sb, t0, tn, 2, 128)
            nc.vector.tensor_tensor(dst, in0, in1, mult)

        emit_s(0)  # task-0 S first: it gates the first main matmul
        emit_t_chunk(0)
        # keep the PE busy across the S->main gap; reading w1t_sb makes these
        # depend on the input DMA so the scheduler cannot hoist them above
        # the S matmuls (which would delay the tanh chain behind them)
        for _ in range(5):
            dps = psum.tile([128, GROUP], _F32, tag="ps")
            nc.tensor.matmul(
                dps[:, :512],
                lhsT=w1t_sb[:, :128],
                rhs=inp_sb[:, :512],
                start=True,
                stop=True,
            )
        for g in range(1, NCHUNK):
            emit_t_chunk(g)
        emit_s(1)
        emit_s(2)
        TTf = TT.rearrange("p k t a -> p k (t a)")  # [128, 2, P]

        # ---- main: H[b, (t,a)] = sum_k S[k,b] T[k,(t,a)] --------------------
        # DMA queues per (task, pair-of-groups); DGE transfers carry ~1.1us
        # fixed overhead each, so ship 2048-col pairs over three queues.
        DQ = (
            (nc.sync, nc.gpsimd, nc.scalar, nc.gpsimd),
            (nc.gpsimd, nc.sync, nc.scalar, nc.sync),
            (nc.sync, nc.gpsimd, nc.gpsimd, nc.sync),
        )
        for t in range(NTASK):
            stg = stage_pool.tile([128, P], mmdt)
            for n in range(NGRP):
                ps = psum.tile([128, GROUP], _F32, tag="ps")
                o = n * GROUP
                for kc in range(2):  # kc-outer: reuse lhsT across the group
                    for h in range(GROUP // 512):
                        nc.tensor.matmul(
                            ps[:, h * 512 : (h + 1) * 512],
                            lhsT=s_sb[:, t, kc, :],
                            rhs=TTf[:, kc, o + h * 512 : o + (h + 1) * 512],
                            start=(kc == 0),
                            stop=(kc == 1),
                        )
                dst = stg[:, o : o + GROUP]
                if t == NTASK - 1 and n == NGRP - 1:
                    # final group: split the drain across both engines
                    nc.vector.tensor_copy(out=dst[:, :512], in_=ps[:, :512])
                    nc.scalar.copy(dst[:, 512:], ps[:, 512:])
                elif n in VE_N[t]:
                    nc.vector.tensor_copy(out=dst, in_=ps)
                else:
                    nc.scalar.copy(dst, ps)
                if n % 2 == 1 and n < NGRP - 1:
                    o2 = (n - 1) * GROUP
                    DQ[t][n // 2].dma_start(
                        out16[t][:, o2 : o2 + 2 * GROUP], stg[:, o2 : o2 + 2 * GROUP]
                    )
            # tail columns
            pst = psum.tile([128, GROUP], _F32, tag="ps")
            o = NGRP * GROUP
            for kc in range(2):
                nc.tensor.matmul(
                    pst[:, :TAIL],
                    lhsT=s_sb[:, t, kc, :],
                    rhs=TTf[:, kc, o:P],
                    start=(kc == 0),
                    stop=(kc == 1),
                )
            (nc.vector.tensor_copy(out=stg[:, o:P], in_=pst[:, :TAIL])
             if t == NTASK - 1 else nc.scalar.copy(stg[:, o:P], pst[:, :TAIL]))
            o2 = (NGRP - 2) * GROUP
            DQ[t][3].dma_start(out16[t][:, o2:], stg[:, o2:])


_NC_CACHE = {}


def _core_tasks(c):
    i = c // 2
    js = [j for j in range(N) if j != i]
    halves = [(j, h) for j in js for h in (0, 1)]
    return i, (halves[0:3] if c % 2 == 0 else halves[3:6])


def _np_mmdt():
    return np.dtype("bfloat16") if MM_MODE == "bf16" else np.float16


def _build():
    key = "fp16" if MM_MODE != "bf16" else "bf16"
    if key in _NC_CACHE:
        return _NC_CACHE[key]
    mmdt = _mmdt()
    nc = bacc.Bacc("TRN2", target_bir_lowering=False, debug=False, num_devices=NCORES)
    inp = nc.dram_tensor("inp", [128, NCOLS], mmdt, kind="ExternalInput").ap()
    b1c = nc.dram_tensor("b1c", [128, 2], _F32, kind="ExternalInput").ap()
    out16 = nc.dram_tensor("out16", [NTASK, HALF, P], mmdt, kind="ExternalOutput").ap()
    with tile.TileContext(nc) as tc:
        _emit(tc, nc, inp, b1c, out16)
    nc.compile()
    _NC_CACHE[key] = nc
    return nc


def _unpack_index():
    """g[a*128+c] = packed column (t*128 + row) holding H[a, c]."""
    a = np.arange(128)[:, None]
    c = np.arange(128)[None, :]
    d = (c - a) % 128
    t = np.where(d <= 64, d, 128 - d)
    row = np.where(d <= 64, a, c)
    return (t * 128 + row).ravel()


_G_IDX = _unpack_index()


# Options for test harness introspection (set by test.py, unused in grading).
_RUN_KWARGS = {}
_LAST_RESULT = None


def kernel(z_all, W1, b1, W2, b2):
    global _LAST_RESULT
    z_all = np.asarray(z_all, dtype=np.float32)
    W1 = np.asarray(W1, dtype=np.float32)
    b1 = np.asarray(b1, dtype=np.float32)
    W2 = np.asarray(W2, dtype=np.float32)

    nc = _build()
    mdt = _np_mmdt()

    in_maps = []
    metas = []
    for c in range(NCORES):
        i, tasks = _core_tasks(c)
        metas.append((i, tasks))
        w1a = W1[i].reshape(2, 128, 128).transpose(1, 0, 2)  # [k%128, kc, a]
        w1d = np.concatenate([w1a, w1a[:, :, :66]], axis=2)  # [k%128, kc, 194]
        w1s = (-2.0 * W2[i, 0])[:, None] * W1[i]  # [256, 128]
        w1s = w1s.reshape(2, 128, 128).transpose(1, 0, 2)
        ztd = np.stack(
            [z_all[j, h * HALF : (h + 1) * HALF, :].T for (j, h) in tasks], axis=1
        )  # [d, task, b]
        inp = np.concatenate(
            [
                W1[i].T,
                ztd.reshape(128, -1),
                w1d.reshape(128, -1),
                w1s.reshape(128, -1),
            ],
            axis=1,
        )
        assert inp.shape == (128, NCOLS), inp.shape
        in_maps.append(
            {
                "inp": np.ascontiguousarray(inp).astype(mdt),
                "b1c": np.ascontiguousarray(b1[i].reshape(2, 128).T),
            }
        )

    res = run_bass_kernel_spmd(nc, in_maps, list(range(NCORES)), **_RUN_KWARGS)
    _LAST_RESULT = res

    full = np.zeros((N, N, B, D, D), dtype=np.float32)
    fullv = full.reshape(N, N, B, D * D)
    for c in range(NCORES):
        i, tasks = metas[c]
        packed = np.asarray(res.results[c]["out16"]).astype(np.float32)
        for t, (j, h) in enumerate(tasks):
            fullv[i, j, h * HALF : (h + 1) * HALF] = packed[t][:, _G_IDX]
    return full
